# revision 85
# baseline (speedup 1.0000x reference)
"""GQA causal attention block (RMSNorm+RoPE+gain, flash-style) on 8 Trainium2 cores.

Problem: nn_Attention (B=2, S=2048, D=1024, H=16, KVH=4, HD=64), fp32 in/out.

Sharding: core c = (b, g) with b = c//4 (batch), g = c%4 (kv-head group).
Each core computes q-heads 4g..4g+3 and kv-head g for batch b, runs causal
attention for its 4 heads, and produces the partial wo product
  part_c = y_c @ wo[:, 256g:256g+256].T   in [2048, 1024] (bf16)
The host sums the 4 partials per batch in fp32.

v2 design (vs baseline):
 - bf16 on the whole attention path (x, w, q, k, v, attention weights, y,
   wo, partial outputs; fp32 norm stats / PSUM accumulation).
 - Scores matmuls trimmed to the causal region; exp merged per sk-tile
   PAIR over a 4-slot PSUM ring built as two 2-slot halves so the Tile
   framework's tile-granularity dependency tracking stays exact per pair.
 - Scores emitted two pairs ahead of exp/AV so ACT runs back-to-back.
 - q/k transposed via HWDGE DMA-transpose (XBAR) straight into SBUF
   (no PE transposes, no PSUM, no evac copies); per stile 4 transposes
   give q0/q2 at base partition 0, q1/q3 at base 64, and kT at both
   bases so matmul operand base partitions always match.
 - v1 carries 64 ones-columns so the AV matmul replicates the softmax
   denominator into po rows 64:128 (free: matmul cost is N-only);
   normalization = one DVE reciprocal over the replica + one multiply.
 - Batched DMAs (one per x chunk / weight block / output stile) to
   amortize the ~625ns per-DMA HWDGE descriptor generation.
 - Output partial written bf16; final chunk split by head-pair so wo
   work streams during the last heads; host sums partials in fp32.
 - Fine-grained emission interleaving: projection / wo ops fill PE gaps
   inside the attention sk loop via two filler queues; latency-critical
   ops (norm stats, causal masks, head tails) carry scheduler priority.
 - Three attention-weight SBUF halves (one more than the PSUM score
   ring) decouple exp from trailing AV reads; work pool triple-buffered.

Engine constraints honored (walrus): GPSIMD never touches PSUM; no DVE
divide; max one PSUM operand per vector op; SB+SB operands share a base
partition; >1 sem wait per instruction split onto NoOps.

PSUM bank map (8 banks x 2KB):
  b0   : proj [128,384] f32 (chunk-0 odd stiles borrow po slots)
  b1-4 : score ring, two [128, 2*512] f32 halves
  b5-6 : po [128,512] f32 x2 (AV accumulator ring over heads; rows
         64:128 = denominator replica)
  b7   : pw [128,512] f32 (wo accumulator; final-chunk t1 units rotate
         through the freed po slots)
"""

import os
import sys

sys.path.insert(0, "/opt/trn_rl_repo")

from collections import deque

import numpy as np
import ml_dtypes
import concourse.bass as bass
import concourse.mybir as mybir
import concourse.tile as tile
from concourse.bass_utils import run_bass_kernel_spmd

F32 = mybir.dt.float32
BF16 = mybir.dt.bfloat16
AL = mybir.AluOpType
AF = mybir.ActivationFunctionType

B, S, D = 2, 2048, 1024
H, KVH, HD = 16, 4, 64
G = H // KVH          # q heads per core (= per kv head)
NC = 8
ST = 128              # s-tile rows
NST = S // ST         # 16
KT = 128              # contraction tile
NKT = D // KT         # 8
SQC = 512             # sq chunk width in attention
NSQC = S // SQC       # 4
CT = SQC // ST        # s-tiles per chunk (4)
NH5 = G + 1           # 4 q heads + 1 k head share norm/rope
ROPE_BASE = 10000.0
EPS = float(np.finfo(np.float32).eps)

LAST_EXEC_NS = None

_counter = [0]

# debug labeling for trace tooling: trace_window.py wraps add_instruction and
# reads DBG_LABEL at emission time; harmless in normal runs
DBG_LABEL = ["?"]


def _lbl(s):
    DBG_LABEL[0] = s


def _split_waits(nc, cap=1):
    """Walrus in this toolchain rejects >1 sync wait per instruction; hoist
    extras onto same-engine NoOps."""
    n = 0
    for f in nc.m.functions:
        for blk in f.blocks:
            out = []
            for inst in blk.instructions:
                si = inst.sync_info
                if si is not None and si.on_wait and len(si.on_wait) > cap:
                    waits = list(si.on_wait)
                    extra, keep = waits[:-cap], waits[-cap:]
                    for w in extra:
                        _counter[0] += 1
                        out.append(
                            mybir.InstNoOp(
                                name=f"WSPLIT-{_counter[0]}",
                                engine=inst.engine,
                                ins=[],
                                outs=[],
                                sync_info=mybir.SyncInfo(on_wait=[w], on_update=[]),
                            )
                        )
                    inst.sync_info = mybir.SyncInfo(
                        on_wait=keep, on_update=list(si.on_update)
                    )
                    n += 1
                out.append(inst)
            blk.instructions[:] = out
    return n


def build_nc(reps=1):
    nc = bass.Bass("TRN2", target_bir_lowering=False, debug=False, num_devices=NC)

    xt_d = nc.dram_tensor("xt", [D, S], BF16, kind="ExternalInput").ap()
    wt_d = nc.dram_tensor("wt", [D, 384], BF16, kind="ExternalInput").ap()
    wot_d = nc.dram_tensor("wot", [G * HD, D], BF16, kind="ExternalInput").ap()
    cosd_d = nc.dram_tensor("cosd", [ST, NST * HD], BF16, kind="ExternalInput").ap()
    sind_d = nc.dram_tensor("sind", [ST, NST * HD], BF16, kind="ExternalInput").ap()
    gains_d = nc.dram_tensor("gains", [ST, 8], F32, kind="ExternalInput").ap()
    mask_d = nc.dram_tensor("mask", [ST, ST], BF16, kind="ExternalInput").ap()
    eps_d = nc.dram_tensor("epsc", [ST, 1], F32, kind="ExternalInput").ap()
    part0_d = nc.dram_tensor("part0", [S, D], BF16, kind="ExternalOutput").ap()
    part1_d = nc.dram_tensor("part1", [S, D], BF16, kind="ExternalOutput").ap()

    with tile.TileContext(nc) as tc:
        with (
            nc.allow_low_precision(reason="bf16 attention path"),
            tc.tile_pool(name="persist", bufs=1) as pp,
            tc.tile_pool(name="xpool", bufs=2) as px,
            tc.tile_pool(name="work", bufs=3) as pw_,
            tc.tile_pool(name="obuf", bufs=3) as pob,
            tc.tile_pool(name="pspersist", bufs=1, space="PSUM") as ppsp,
            tc.tile_pool(name="ps_b0", bufs=1, space="PSUM") as ps_b0,
            tc.tile_pool(name="ps_po", bufs=2, space="PSUM") as ps_po,
            tc.tile_pool(name="ps_w", bufs=1, space="PSUM") as ps_w,
        ):
            # ---- persistent SBUF ----
            # DMA-transposed q/k. qkv column layout is [q0|q1|q2|k|q3], so
            # three overlapping 128-col transposes cover everything with k
            # landing at BOTH base partitions:
            #   qt0 = T(cols   0:128): rows 0:64 = q0T, 64:128 = q1T
            #   qt1 = T(cols 128:256): rows 0:64 = q2T, 64:128 = kT
            #   qt2 = T(cols 192:320): rows 0:64 = kT,  64:128 = q3T
            qtT = [
                [pp.tile([ST, SQC], BF16, tag=f"qt{i}_{qc}", name=f"qt{i}_{qc}")
                 for i in range(3)]
                for qc in range(NSQC)
            ]
            _qmap = [(0, 0), (0, HD), (1, 0), (2, HD)]  # head -> (tile, base)
            qTc = [
                [qtT[qc][_qmap[h][0]][_qmap[h][1]:_qmap[h][1] + HD, :]
                 for qc in range(NSQC)]
                for h in range(G)
            ]
            # per-head kT view with matching base partition (h even -> base 0)
            kTc = [
                [qtT[qc][2][0:HD, :], qtT[qc][1][HD:ST, :]]
                for qc in range(NSQC)
            ]
            v1c = [pp.tile([ST, CT * ST], BF16, tag=f"v1{qc}", name=f"v1c{qc}")
                   for qc in range(NSQC)]
            ypc = [
                [pp.tile([ST, SQC], BF16, tag=f"yp{t}_{qc}", name=f"yp{t}_{qc}")
                 for qc in range(NSQC)]
                for t in range(2)
            ]
            wotp_t = pp.tile([ST, 2 * D], BF16, tag="wot", name="wotp")
            wotp = [wotp_t[:, t * D:(t + 1) * D] for t in range(2)]
            cosd = pp.tile([ST, NST * HD], BF16, tag="cosd")
            sind = pp.tile([ST, NST * HD], BF16, tag="sind")
            gains = pp.tile([ST, 8], F32, tag="gains")
            maskt = pp.tile([ST, ST], BF16, tag="mask")
            epst = pp.tile([ST, 1], F32, tag="eps")
            wts_t = pp.tile([KT, NKT * 384], BF16, tag="wt", name="wts")
            wts = [wts_t[:, k * 384:(k + 1) * 384] for k in range(NKT)]
            # exp output ring: three 2-slot halves (one more than the psum
            # ring) so exp(p+2) does not wait on pair p's AV reads
            atrh = [pp.tile([ST, 2 * SQC], BF16, tag=f"atr{i}", name=f"atr{i}")
                    for i in range(3)]
            # psum score ring: 2+2 banks (pair p lives entirely in one half)
            pscrh = [ppsp.tile([ST, 2 * SQC], F32, tag=f"pscr{i}", name=f"pscr{i}")
                     for i in range(2)]

            # ---- input DMAs (SP queue), proj-critical first ----
            xtc_map = {}

            def emit_xtc(qc):
                xt_t = px.tile([KT, NKT * SQC], BF16, tag="xt", name=f"xtc_{qc}")
                nc.sync.dma_start(
                    out=xt_t[:].rearrange("p (k c) -> p k c", c=SQC),
                    in_=xt_d.rearrange("(k p) c -> p k c", p=KT)
                        [:, :, qc * SQC:(qc + 1) * SQC],
                )
                xtc_map[qc] = [xt_t[:, k * SQC:(k + 1) * SQC] for k in range(NKT)]
                # ones half-columns of v1 for this chunk (strided memset)
                v1g = v1c[qc][:].rearrange("p (m c) -> p m c", c=ST)[:, :, HD:ST]
                nc.vector.memset(v1g, 1.0)

            # startup loads: interleave wt/x piecewise so the first
            # projection k-steps can begin as soon as their slices land
            # (the DMA pipe serializes transfers; mega-DMAs are all-or-nothing)
            wt_r = wt_d.rearrange("(k p) c -> p k c", p=KT)
            wts_r = wts_t[:].rearrange("p (k c) -> p k c", c=384)
            xt0_t = px.tile([KT, NKT * SQC], BF16, tag="xt", name="xtc_0")
            xt0_r = xt0_t[:].rearrange("p (k c) -> p k c", c=SQC)
            xt_r = xt_d.rearrange("(k p) c -> p k c", p=KT)
            nc.scalar.dma_start(out=wts_r[:, 0:2], in_=wt_r[:, 0:2])
            nc.sync.dma_start(out=xt0_r[:, 0:2], in_=xt_r[:, 0:2, 0:SQC])
            nc.scalar.dma_start(out=wts_r[:, 2:5], in_=wt_r[:, 2:5])
            nc.sync.dma_start(out=xt0_r[:, 2:5], in_=xt_r[:, 2:5, 0:SQC])
            nc.scalar.dma_start(out=wts_r[:, 5:8], in_=wt_r[:, 5:8])
            nc.sync.dma_start(out=xt0_r[:, 5:8], in_=xt_r[:, 5:8, 0:SQC])
            xtc_map[0] = [xt0_t[:, k * SQC:(k + 1) * SQC] for k in range(NKT)]
            v1g0 = v1c[0][:].rearrange("p (m c) -> p m c", c=ST)[:, :, HD:ST]
            nc.vector.memset(v1g0, 1.0)
            # initialize the score ring: merged diag exps read whole halves,
            # including columns no scores matmul has written yet
            nc.vector.memset(pscrh[0][:], 0.0)
            nc.vector.memset(pscrh[1][:], 0.0)
            nc.scalar.dma_start(out=epst[:], in_=eps_d[:])
            nc.scalar.dma_start(out=gains[:], in_=gains_d[:])
            nc.scalar.dma_start(out=cosd[:], in_=cosd_d[:])
            nc.scalar.dma_start(out=sind[:], in_=sind_d[:])
            nc.scalar.dma_start(out=maskt[:], in_=mask_d[:])
            nc.scalar.dma_start(
                out=wotp_t[:].rearrange("p (t c) -> p t c", c=D),
                in_=wot_d.rearrange("(t p) c -> p t c", p=ST),
            )

            # ---------------- emission helpers ----------------
            # fq1: phase-1 ops (must drain before the next chunk's heads)
            # fq3: wo/output ops (can spill across chunks)
            fq1 = deque()
            fq3 = deque()
            flip = [0]

            def fill(n=1):
                for _ in range(n):
                    flip[0] += 1
                    qa, qb = (fq3, fq1) if flip[0] % 2 == 0 else (fq1, fq3)
                    if qa:
                        qa.popleft()()
                    elif qb:
                        qb.popleft()()
                    else:
                        return

            def drain_p1():
                while fq1:
                    fq1.popleft()()

            def drain_all():
                drain_p1()
                while fq3:
                    fq3.popleft()()

            # ---- phase 1: projection + stats + rope + transposes for a chunk
            def p1_ops(qc):
                """Return list of emission closures for phase-1 of chunk qc."""
                ops = []
                xtc = xtc_map  # read lazily; xtc[qc] created by emit_xtc op
                ss = pw_.tile([ST, 32], F32, tag="ss", name=f"ss{qc}")
                rr = pw_.tile([ST, 32], F32, tag="rr", name=f"rr{qc}")
                rg = pw_.tile([ST, 32], F32, tag="rg", name=f"rg{qc}")
                qkes = []

                def proj_mm(s, k0):
                    def op():
                        _lbl(f"proj{qc}.{s}.{k0}")
                        if k0 == 0:
                            if qc == 0 and s % 2 == 1:
                                proj = ps_po.tile([ST, 384], F32, tag="po",
                                                  name=f"pj{qc}_{s}",
                                                  padded_shape=[ST, SQC])
                            else:
                                proj = ps_b0.tile([ST, 384], F32, tag="b0",
                                                  name=f"pj{qc}_{s}",
                                                  padded_shape=[ST, SQC])
                            qkes.append(proj)
                        proj = qkes[s]
                        for k in range(k0, k0 + 2):
                            nc.tensor.matmul(
                                proj[:],
                                xtc[qc][k][:, s * ST:(s + 1) * ST],
                                wts[k][:],
                                start=(k == 0),
                                stop=(k == NKT - 1),
                            )
                    op.is_pe = True
                    return op

                qke_t = []

                def evac(s):
                    def op():
                        _lbl(f"evac{qc}.{s}")
                        proj = qkes[s]
                        qke = pw_.tile([ST, 384], BF16, tag="qke", name=f"qke{qc}_{s}",
                                       bufs=8)
                        qke_t.append(qke)
                        nc.vector.tensor_copy(qke[:], proj[:])
                        nc.gpsimd.tensor_copy(
                            v1c[qc][:, s * ST:s * ST + HD],
                            qke[:, 320:384],
                        )
                    return op

                def stats(s):
                    def op():
                        _lbl(f"stats{qc}.{s}")
                        qke = qke_t[s]
                        sq = pw_.tile([ST, 320], BF16, tag="sq", name=f"sq{qc}_{s}")
                        nc.gpsimd.tensor_tensor(sq[:], qke[:, 0:320], qke[:, 0:320],
                                                AL.mult)
                        nc.vector.tensor_reduce(
                            ss[:, 8 * s:8 * s + NH5],
                            sq[:].rearrange("p (h d) -> p h d", d=HD),
                            axis=mybir.AxisListType.X,
                            op=AL.add,
                        )
                    return op

                def chunk_stats():
                    _lbl(f"cstats{qc}")
                    # rstd*gain = exp(-0.5*ln(ms+eps)) * gain, batched over 4
                    # stiles. High priority: these two tiny ACT ops gate the
                    # whole rope -> transpose chain and must not queue behind
                    # attention exps.
                    lg = pw_.tile([ST, 32], F32, tag="lg", name=f"lg{qc}")
                    sv = ss[:].rearrange("p (s c) -> p s c", c=8)[:, :, 0:NH5]
                    lv = lg[:].rearrange("p (s c) -> p s c", c=8)[:, :, 0:NH5]
                    rv = rr[:].rearrange("p (s c) -> p s c", c=8)[:, :, 0:NH5]
                    gv = rg[:].rearrange("p (s c) -> p s c", c=8)[:, :, 0:NH5]
                    with tc.high_priority(offset=400):
                        nc.scalar.activation(lv, sv, AF.Ln, bias=epst[:, 0:1],
                                             scale=1.0 / HD)
                        nc.scalar.activation(rv, lv, AF.Exp, scale=-0.5)
                        nc.vector.tensor_tensor(
                            gv, rv,
                            gains[:, 0:NH5].unsqueeze(1).broadcast_to([ST, 4, NH5]),
                            AL.mult,
                        )

                def rope_a(s):
                    def op():
                        _lbl(f"rope{qc}.{s}")
                        m = qc * CT + s
                        qke = qke_t[s]
                        qke3 = qke[:, 0:320].rearrange("p (h d) -> p h d", d=HD)
                        cosm = cosd[:, m * HD:(m + 1) * HD]
                        sinm = sind[:, m * HD:(m + 1) * HD]
                        tcc = pw_.tile([ST, 320], BF16, tag="tcc", name=f"tc{qc}_{s}")
                        nc.vector.tensor_tensor(
                            tcc[:].rearrange("p (h d) -> p h d", d=HD),
                            qke3,
                            cosm.unsqueeze(1).broadcast_to([ST, NH5, HD]),
                            AL.mult,
                        )
                        tss = pw_.tile([ST, 320], BF16, tag="tss", name=f"ts{qc}_{s}")
                        tss3 = tss[:].rearrange("p (h d) -> p h d", d=HD)
                        HH = HD // 2
                        nc.vector.tensor_tensor(
                            tss3[:, :, 0:HH],
                            qke3[:, :, HH:HD],
                            sinm[:, 0:HH].unsqueeze(1).broadcast_to([ST, NH5, HH]),
                            AL.mult,
                        )
                        nc.vector.tensor_tensor(
                            tss3[:, :, HH:HD],
                            qke3[:, :, 0:HH],
                            sinm[:, HH:HD].unsqueeze(1).broadcast_to([ST, NH5, HH]),
                            AL.mult,
                        )
                        qkrr = pw_.tile([ST, 320], BF16, tag="qkrr", name=f"qr{qc}_{s}")
                        nc.vector.tensor_tensor(qkrr[:], tcc[:], tss[:], AL.add)
                        qkr = pw_.tile([ST, 320], BF16, tag="qkr", name=f"qk{qc}_{s}")
                        nc.vector.tensor_tensor(
                            qkr[:, 0:320].rearrange("p (h d) -> p h d", d=HD),
                            qkrr[:].rearrange("p (h d) -> p h d", d=HD),
                            rg[:, 8 * s:8 * s + NH5].unsqueeze(2)
                              .broadcast_to([ST, NH5, HD]),
                            AL.mult,
                        )
                        qkr_t.append(qkr)
                    return op

                qkr_t = []

                def tr_dma(s, i):
                    def op():
                        _lbl(f"trq{qc}.{s}.{i}")
                        qkr = qkr_t[s]
                        c0 = (0, 128, 192)[i]
                        nc.sync.dma_start(
                            out=qtT[qc][i][:, s * ST:(s + 1) * ST],
                            in_=qkr[:, c0:c0 + ST],
                            transpose=True,
                        )
                    return op

                for s in range(CT):
                    for k0 in range(0, NKT, 2):
                        ops.append(proj_mm(s, k0))
                    ops.append(evac(s))
                    ops.append(stats(s))
                ops.append(chunk_stats)
                # per-stile A/D transposes fire immediately after their own
                # rope (head0 needs A+D); C then B follow for heads 1-3.
                # HWDGE generation is serialized, so issue order = first use.
                for s in range(CT):
                    ops.append(rope_a(s))
                    ops.append(tr_dma(s, 0))
                    ops.append(tr_dma(s, 2))
                for s in range(CT):
                    ops.append(tr_dma(s, 1))
                return ops

            # ---- phase 3: wo matmuls + evac + output DMA ----
            # chunks 0..2: both head-pairs accumulated into one psum (part0).
            # final chunk: split by head-pair t so t=0 streams after head 1;
            # the t=1 tail rotates through the freed po banks (part1).
            wo_po_alt = [0]

            def p3_ops(qc, mode):
                if mode == "t1":
                    wo_po_alt[0] = 0
                ops = []

                obs = {}

                def half(s, nch, ts, part_t, use_po):
                    def op():
                        _lbl(f"wo{qc}.{s}.{nch}.{mode}")
                        m = qc * CT + s
                        if use_po and wo_po_alt[0] % 3 != 0:
                            pw = ps_po.tile([ST, SQC], F32, tag="po",
                                            name=f"pwp{qc}_{s}_{nch}")
                        else:
                            pw = ps_w.tile([ST, SQC], F32, tag="pw",
                                           name=f"pw{qc}_{s}_{nch}")
                        if use_po:
                            wo_po_alt[0] += 1
                        for i, t in enumerate(ts):
                            nc.tensor.matmul(
                                pw[:],
                                ypc[t][qc][:, s * ST:(s + 1) * ST],
                                wotp[t][:, nch * SQC:(nch + 1) * SQC],
                                start=(i == 0),
                                stop=(i == len(ts) - 1),
                            )
                        if nch == 0:
                            ob = pob.tile([ST, D], BF16, tag="ob",
                                          name=f"ob{qc}_{s}_{mode}", bufs=4)
                            obs[s] = ob
                        else:
                            ob = obs[s]
                        o0 = nch * SQC
                        if mode == "both":
                            nc.vector.tensor_copy(ob[:, o0:o0 + SQC], pw[:])
                        else:
                            nc.vector.tensor_copy(ob[:, o0:o0 + 256], pw[:, 0:256])
                            nc.scalar.activation(ob[:, o0 + 256:o0 + SQC],
                                                 pw[:, 256:SQC], AF.Copy)
                        if nch == 1:
                            nc.sync.dma_start(
                                out=part_t[m * ST:(m + 1) * ST, :],
                                in_=ob[:],
                            )
                    op.is_pe = True
                    return op

                if mode == "both":
                    ts, part_t, use_po = (0, 1), part0_d, False
                elif mode == "t0":
                    ts, part_t, use_po = (0,), part0_d, False
                else:
                    ts, part_t, use_po = (1,), part1_d, True
                for s in range(CT):
                    ops.append(half(s, 0, ts, part_t, use_po))
                    ops.append(half(s, 1, ts, part_t, use_po))
                return ops

            # ---- attention head ----
            ring = [0]   # score-ring cursor (4 psum slots; pairs aligned)
            aring = [0]  # atr-half cursor (3 sbuf halves)

            def emit_attn_head(qc, h):
                nsk = (qc + 1) * CT
                if qc == 0 and h == 2:
                    # pw bank is idle before any wo work: extra po slot
                    po = ps_w.tile([ST, SQC], F32, tag="pw", name=f"po{qc}_{h}")
                else:
                    po = ps_po.tile([ST, SQC], F32, tag="po", name=f"po{qc}_{h}")
                qT = qTc[h][qc]
                info = []
                for sk in range(nsk):
                    slot = ring[0]
                    ring[0] = (ring[0] + 1) % 4
                    dj = sk - qc * CT
                    cb = dj * ST if dj >= 0 else 0
                    info.append((slot, cb))
                pair_af = []
                for p in range(nsk // 2):
                    pair_af.append(aring[0])
                    aring[0] = (aring[0] + 1) % 3

                def scores(sk):
                    _lbl(f"sc{qc}.{h}.{sk}")
                    slot, cb = info[sk]
                    hf, sub = slot // 2, slot % 2
                    skc, skm = sk // CT, sk % CT
                    # kT view whose base partition matches qT's (h parity)
                    kT = kTc[skc][h % 2]
                    nc.tensor.matmul(
                        pscrh[hf][:, sub * SQC + cb:(sub + 1) * SQC],
                        kT[:, skm * ST:(skm + 1) * ST],
                        qT[:, cb:SQC],
                        start=True,
                        stop=True,
                    )

                def exp_pair(p):
                    _lbl(f"exp{qc}.{h}.{p}")
                    sk0 = 2 * p
                    hf = info[sk0][0] // 2
                    af = pair_af[p]
                    diag = sk0 >= qc * CT
                    if not diag or qc <= 3:
                        # one exp over the whole ring half; for diag pairs the
                        # left-of-cb columns hold stale-but-bounded scores and
                        # are never read by the trimmed AV (chunk 3 keeps the
                        # trimmed singles: ACT is the pacer there)
                        nc.scalar.activation(
                            atrh[af][:], pscrh[hf][:], AF.Exp,
                        )
                        if diag:
                            for sk in (sk0, sk0 + 1):
                                slot, cb = info[sk]
                                sub = slot % 2
                                with tc.high_priority(offset=300):
                                    nc.vector.tensor_tensor(
                                        atrh[af][:, sub * SQC + cb:sub * SQC + cb + ST],
                                        atrh[af][:, sub * SQC + cb:sub * SQC + cb + ST],
                                        maskt[:],
                                        AL.mult,
                                    )
                    else:
                        for sk in (sk0, sk0 + 1):
                            slot, cb = info[sk]
                            sub = slot % 2
                            nc.scalar.activation(
                                atrh[af][:, sub * SQC + cb:(sub + 1) * SQC],
                                pscrh[hf][:, sub * SQC + cb:(sub + 1) * SQC],
                                AF.Exp,
                            )
                            with tc.high_priority(offset=300):
                                nc.vector.tensor_tensor(
                                    atrh[af][:, sub * SQC + cb:sub * SQC + cb + ST],
                                    atrh[af][:, sub * SQC + cb:sub * SQC + cb + ST],
                                    maskt[:],
                                    AL.mult,
                                )

                def av(sk):
                    _lbl(f"av{qc}.{h}.{sk}")
                    slot, cb = info[sk]
                    sub = slot % 2
                    af = pair_af[sk // 2]
                    skc, skm = sk // CT, sk % CT
                    nc.tensor.matmul(
                        po[:, cb:SQC],
                        v1c[skc][:, skm * ST:(skm + 1) * ST],
                        atrh[af][:, sub * SQC + cb:(sub + 1) * SQC],
                        start=(sk == 0),
                        stop=(sk == nsk - 1),
                    )

                # software-pipelined: scores run two pairs ahead so exp(p+1)'s
                # inputs complete during exp(p) and ACT goes back-to-back.
                # s(2p+4) reuses exp(p)'s input slot, so it must be emitted
                # after exp_pair(p) (program order creates the WAR dep).
                for sk in range(min(4, nsk)):
                    scores(sk)
                    fill(1)
                for p in range(nsk // 2):
                    exp_pair(p)
                    if 2 * p + 4 < nsk:
                        scores(2 * p + 4)
                        fill(1)
                        scores(2 * p + 5)
                        fill(1)
                    av(2 * p)
                    fill(1)
                    av(2 * p + 1)
                    fill(1)
                # tail: po rows 64:128 all hold the denominator (v1 ones
                # half); reciprocal over the replica = broadcast 1/den.
                # high priority: frees the po slot for head h+2 sooner.
                _lbl(f"tail{qc}.{h}")
                rb = pw_.tile([HD, SQC], F32, tag="rb", name=f"rb{qc}_{h}")
                with tc.high_priority(offset=200):
                    nc.vector.reciprocal(rb[:], po[HD:ST, :])
                    t, r = h // 2, (h % 2) * HD
                    nc.vector.tensor_tensor(
                        ypc[t][qc][r:r + HD, :], po[0:HD, :], rb[:], AL.mult
                    )

            # ---------------- main emission ----------------
            for rep in range(reps):
                # chunk 0 phase 1, emitted directly (nothing to interleave
                # with); chunk 1's projection/stats stages follow eagerly so
                # the b0-slot chain overlaps chunk-0's rope instead of its
                # attention, leaving rope/transposes as filler
                for op in p1_ops(0):
                    op()
                emit_xtc(1)
                o1 = p1_ops(1)
                for op in o1[:19]:
                    op()
                for op in o1[19:]:
                    fq1.append(op)
                for qc in range(NSQC):
                    if 2 <= qc + 1 < NSQC:
                        emit_xtc(qc + 1)
                        for op in p1_ops(qc + 1):
                            fq1.append(op)
                    if qc >= 1:
                        for op in p3_ops(qc - 1, "both"):
                            fq3.append(op)
                    last = qc == NSQC - 1
                    for h in range(G):
                        emit_attn_head(qc, h)
                        if last and h == 1:
                            for op in p3_ops(qc, "t0"):
                                fq3.append(op)
                        elif last and h == 3:
                            for op in p3_ops(qc, "t1"):
                                fq3.append(op)
                        # keep phase-1 filler on pace across the chunk's heads
                        heads_left = G - h - 1
                        if heads_left:
                            target = (len(fq1) + heads_left - 1) // heads_left
                            extra = len(fq1) - target * heads_left
                            while extra > 0 and fq1:
                                fq1.popleft()()
                                extra -= 1
                        else:
                            drain_p1()
                drain_all()
    return nc


def _host_tables():
    inv_freq = 1.0 / (ROPE_BASE ** (np.arange(0, HD, 2, dtype=np.float32) / HD))
    pos = np.arange(S, dtype=np.float32)
    freqs = np.outer(pos, inv_freq)  # [S, 32]
    cos = np.cos(freqs).astype(np.float32)
    sin = np.sin(freqs).astype(np.float32)
    cosdup = np.concatenate([cos, cos], axis=1)        # [S, 64]
    sindup = np.concatenate([sin, -sin], axis=1)       # [S, 64]
    # rearrange [S, 64] -> [128, 16*64] with [p, m*64+j] = v[m*128+p, j]
    def arr(v):
        return np.ascontiguousarray(
            v.reshape(NST, ST, HD).transpose(1, 0, 2).reshape(ST, NST * HD)
        )
    mask = np.tril(np.ones((ST, ST), np.float32)).T  # mask[i,j] = 1 iff i<=j
    return arr(cosdup), arr(sindup), mask


_NC_CACHE = None


def _get_nc():
    global _NC_CACHE
    if _NC_CACHE is None:
        nc = build_nc()
        _split_waits(nc, cap=1)
        _NC_CACHE = nc
    return _NC_CACHE


BF = ml_dtypes.bfloat16


def make_in_maps(x, wq, wk, wv, wo, q_gain):
    x = np.asarray(x, np.float32)
    wq, wk, wv, wo = (np.asarray(a, np.float32) for a in (wq, wk, wv, wo))
    q_gain = np.asarray(q_gain, np.float32)
    cosd, sind, mask = _host_tables()
    epsc = np.full((ST, 1), EPS, np.float32)
    in_maps = []
    for c in range(NC):
        b, g = c // KVH, c % KVH
        xT = np.ascontiguousarray(x[b].T)  # [D, S]
        wq_c = wq[256 * g:256 * (g + 1), :]
        wk_c = wk[HD * g:HD * (g + 1), :]
        wv_c = wv[HD * g:HD * (g + 1), :]
        # column layout [q0 q1 q2 | k | q3 | v]: k sits in the overlap of the
        # two upper transposes so it lands at both base partitions
        wt = np.ascontiguousarray(
            np.concatenate([wq_c[0:192].T, wk_c.T, wq_c[192:256].T, wv_c.T],
                           axis=1)
        )  # [D, 384]
        wot = np.ascontiguousarray(wo[:, 256 * g:256 * (g + 1)].T)  # [256, D]
        gains = np.zeros((ST, 8), np.float32)
        qg = q_gain[G * g:G * (g + 1)] / np.sqrt(HD)
        gains[:, 0:3] = qg[None, 0:3]
        gains[:, 3] = 1.0       # k segment
        gains[:, 4] = qg[3]
        in_maps.append(
            dict(
                xt=xT.astype(BF), wt=wt.astype(BF), wot=wot.astype(BF),
                cosd=cosd.astype(BF), sind=sind.astype(BF), gains=gains,
                mask=mask.astype(BF), epsc=epsc,
            )
        )
    return in_maps


def kernel(x, wq, wk, wv, wo, q_gain):
    global LAST_EXEC_NS
    nc = _get_nc()
    in_maps = make_in_maps(x, wq, wk, wv, wo, q_gain)
    trace = os.environ.get("BASS_KERNEL_TRACE", "") == "1"
    r = run_bass_kernel_spmd(nc, in_maps, list(range(NC)), trace=trace)
    LAST_EXEC_NS = r.exec_time_ns
    parts = []
    for c in range(NC):
        p = np.asarray(r.results[c]["part0"]).astype(np.float32)
        p[-SQC:, :] += np.asarray(
            r.results[c]["part1"][-SQC:, :]).astype(np.float32)
        parts.append(p)
    out = np.stack(
        [sum(parts[0:KVH]), sum(parts[KVH:2 * KVH])], axis=0
    ).astype(np.float32)
    return out


# revision 91
# speedup vs baseline: 1.0040x; 1.0040x over previous
"""GQA causal attention block (RMSNorm+RoPE+gain, flash-style) on 8 Trainium2 cores.

Problem: nn_Attention (B=2, S=2048, D=1024, H=16, KVH=4, HD=64), fp32 in/out.

Sharding: core c = (b, g) with b = c//4 (batch), g = c%4 (kv-head group).
Each core computes q-heads 4g..4g+3 and kv-head g for batch b, runs causal
attention for its 4 heads, and produces the partial wo product
  part_c = y_c @ wo[:, 256g:256g+256].T   in [2048, 1024] (bf16)
The host sums the 4 partials per batch in fp32.

v2 design (vs baseline):
 - bf16 on the whole attention path (x, w, q, k, v, attention weights, y,
   wo, partial outputs; fp32 norm stats / PSUM accumulation).
 - Scores matmuls trimmed to the causal region; exp merged per sk-tile
   PAIR over a 4-slot PSUM ring built as two 2-slot halves so the Tile
   framework's tile-granularity dependency tracking stays exact per pair.
 - Scores emitted two pairs ahead of exp/AV so ACT runs back-to-back.
 - q/k transposed via HWDGE DMA-transpose (XBAR) straight into SBUF
   (no PE transposes, no PSUM, no evac copies); per stile 4 transposes
   give q0/q2 at base partition 0, q1/q3 at base 64, and kT at both
   bases so matmul operand base partitions always match.
 - v1 carries 64 ones-columns so the AV matmul replicates the softmax
   denominator into po rows 64:128 (free: matmul cost is N-only);
   normalization = one DVE reciprocal over the replica + one multiply.
 - Batched DMAs (one per x chunk / weight block / output stile) to
   amortize the ~625ns per-DMA HWDGE descriptor generation.
 - Output partial written bf16; final chunk split by head-pair so wo
   work streams during the last heads; host sums partials in fp32.
 - Fine-grained emission interleaving: projection / wo ops fill PE gaps
   inside the attention sk loop via two filler queues; latency-critical
   ops (norm stats, causal masks, head tails) carry scheduler priority.
 - Three attention-weight SBUF halves (one more than the PSUM score
   ring) decouple exp from trailing AV reads; work pool triple-buffered.

Engine constraints honored (walrus): GPSIMD never touches PSUM; no DVE
divide; max one PSUM operand per vector op; SB+SB operands share a base
partition; >1 sem wait per instruction split onto NoOps.

PSUM bank map (8 banks x 2KB):
  b0   : proj [128,384] f32 (chunk-0 odd stiles borrow po slots)
  b1-4 : score ring, two [128, 2*512] f32 halves
  b5-6 : po [128,512] f32 x2 (AV accumulator ring over heads; rows
         64:128 = denominator replica)
  b7   : pw [128,512] f32 (wo accumulator; final-chunk t1 units rotate
         through the freed po slots)
"""

import os
import sys

sys.path.insert(0, "/opt/trn_rl_repo")

from collections import deque

import numpy as np
import ml_dtypes
import concourse.bass as bass
import concourse.mybir as mybir
import concourse.tile as tile
from concourse.bass_utils import run_bass_kernel_spmd

F32 = mybir.dt.float32
BF16 = mybir.dt.bfloat16
AL = mybir.AluOpType
AF = mybir.ActivationFunctionType

B, S, D = 2, 2048, 1024
H, KVH, HD = 16, 4, 64
G = H // KVH          # q heads per core (= per kv head)
NC = 8
ST = 128              # s-tile rows
NST = S // ST         # 16
KT = 128              # contraction tile
NKT = D // KT         # 8
SQC = 512             # sq chunk width in attention
NSQC = S // SQC       # 4
CT = SQC // ST        # s-tiles per chunk (4)
NH5 = G + 1           # 4 q heads + 1 k head share norm/rope
ROPE_BASE = 10000.0
EPS = float(np.finfo(np.float32).eps)

LAST_EXEC_NS = None

_counter = [0]

# debug labeling for trace tooling: trace_window.py wraps add_instruction and
# reads DBG_LABEL at emission time; harmless in normal runs
DBG_LABEL = ["?"]


def _lbl(s):
    DBG_LABEL[0] = s


def _split_waits(nc, cap=1):
    """Walrus in this toolchain rejects >1 sync wait per instruction; hoist
    extras onto same-engine NoOps."""
    n = 0
    for f in nc.m.functions:
        for blk in f.blocks:
            out = []
            for inst in blk.instructions:
                si = inst.sync_info
                if si is not None and si.on_wait and len(si.on_wait) > cap:
                    waits = list(si.on_wait)
                    extra, keep = waits[:-cap], waits[-cap:]
                    for w in extra:
                        _counter[0] += 1
                        out.append(
                            mybir.InstNoOp(
                                name=f"WSPLIT-{_counter[0]}",
                                engine=inst.engine,
                                ins=[],
                                outs=[],
                                sync_info=mybir.SyncInfo(on_wait=[w], on_update=[]),
                            )
                        )
                    inst.sync_info = mybir.SyncInfo(
                        on_wait=keep, on_update=list(si.on_update)
                    )
                    n += 1
                out.append(inst)
            blk.instructions[:] = out
    return n


def build_nc(reps=1):
    nc = bass.Bass("TRN2", target_bir_lowering=False, debug=False, num_devices=NC)

    xt_d = nc.dram_tensor("xt", [D, S], BF16, kind="ExternalInput").ap()
    wt_d = nc.dram_tensor("wt", [D, 384], BF16, kind="ExternalInput").ap()
    wot_d = nc.dram_tensor("wot", [G * HD, D], BF16, kind="ExternalInput").ap()
    cosd_d = nc.dram_tensor("cosd", [ST, NST * HD], BF16, kind="ExternalInput").ap()
    sind_d = nc.dram_tensor("sind", [ST, NST * HD], BF16, kind="ExternalInput").ap()
    gains_d = nc.dram_tensor("gains", [ST, 8], F32, kind="ExternalInput").ap()
    mask_d = nc.dram_tensor("mask", [ST, ST], BF16, kind="ExternalInput").ap()
    eps_d = nc.dram_tensor("epsc", [ST, 1], F32, kind="ExternalInput").ap()
    part0_d = nc.dram_tensor("part0", [S, D], BF16, kind="ExternalOutput").ap()
    part1_d = nc.dram_tensor("part1", [S, D], BF16, kind="ExternalOutput").ap()

    with tile.TileContext(nc) as tc:
        with (
            nc.allow_low_precision(reason="bf16 attention path"),
            tc.tile_pool(name="persist", bufs=1) as pp,
            tc.tile_pool(name="xpool", bufs=2) as px,
            tc.tile_pool(name="work", bufs=3) as pw_,
            tc.tile_pool(name="obuf", bufs=3) as pob,
            tc.tile_pool(name="pspersist", bufs=1, space="PSUM") as ppsp,
            tc.tile_pool(name="ps_b0", bufs=1, space="PSUM") as ps_b0,
            tc.tile_pool(name="ps_po", bufs=2, space="PSUM") as ps_po,
            tc.tile_pool(name="ps_w", bufs=1, space="PSUM") as ps_w,
        ):
            # ---- persistent SBUF ----
            # DMA-transposed q/k. qkv column layout is [q0|q1|q2|k|q3], so
            # three overlapping 128-col transposes cover everything with k
            # landing at BOTH base partitions:
            #   qt0 = T(cols   0:128): rows 0:64 = q0T, 64:128 = q1T
            #   qt1 = T(cols 128:256): rows 0:64 = q2T, 64:128 = kT
            #   qt2 = T(cols 192:320): rows 0:64 = kT,  64:128 = q3T
            qtT = [
                [pp.tile([ST, SQC], BF16, tag=f"qt{i}_{qc}", name=f"qt{i}_{qc}")
                 for i in range(3)]
                for qc in range(NSQC)
            ]
            _qmap = [(0, 0), (0, HD), (1, 0), (2, HD)]  # head -> (tile, base)
            qTc = [
                [qtT[qc][_qmap[h][0]][_qmap[h][1]:_qmap[h][1] + HD, :]
                 for qc in range(NSQC)]
                for h in range(G)
            ]
            # per-head kT view with matching base partition (h even -> base 0)
            kTc = [
                [qtT[qc][2][0:HD, :], qtT[qc][1][HD:ST, :]]
                for qc in range(NSQC)
            ]
            v1c = [pp.tile([ST, CT * ST], BF16, tag=f"v1{qc}", name=f"v1c{qc}")
                   for qc in range(NSQC)]
            ypc = [
                [pp.tile([ST, SQC], BF16, tag=f"yp{t}_{qc}", name=f"yp{t}_{qc}")
                 for qc in range(NSQC)]
                for t in range(2)
            ]
            wotp_t = pp.tile([ST, 2 * D], BF16, tag="wot", name="wotp")
            wotp = [wotp_t[:, t * D:(t + 1) * D] for t in range(2)]
            cosd = pp.tile([ST, NST * HD], BF16, tag="cosd")
            sind = pp.tile([ST, NST * HD], BF16, tag="sind")
            gains = pp.tile([ST, 8], F32, tag="gains")
            maskt = pp.tile([ST, ST], BF16, tag="mask")
            epst = pp.tile([ST, 1], F32, tag="eps")
            wts_t = pp.tile([KT, NKT * 384], BF16, tag="wt", name="wts")
            wts = [wts_t[:, k * 384:(k + 1) * 384] for k in range(NKT)]
            # exp output ring: three 2-slot halves (one more than the psum
            # ring) so exp(p+2) does not wait on pair p's AV reads
            atrh = [pp.tile([ST, 2 * SQC], BF16, tag=f"atr{i}", name=f"atr{i}")
                    for i in range(3)]
            # psum score ring: 2+2 banks (pair p lives entirely in one half)
            pscrh = [ppsp.tile([ST, 2 * SQC], F32, tag=f"pscr{i}", name=f"pscr{i}")
                     for i in range(2)]

            # ---- input DMAs (SP queue), proj-critical first ----
            xtc_map = {}

            def emit_xtc(qc):
                xt_t = px.tile([KT, NKT * SQC], BF16, tag="xt", name=f"xtc_{qc}")
                nc.sync.dma_start(
                    out=xt_t[:].rearrange("p (k c) -> p k c", c=SQC),
                    in_=xt_d.rearrange("(k p) c -> p k c", p=KT)
                        [:, :, qc * SQC:(qc + 1) * SQC],
                )
                xtc_map[qc] = [xt_t[:, k * SQC:(k + 1) * SQC] for k in range(NKT)]
                # ones half-columns of v1 for this chunk (strided memset)
                v1g = v1c[qc][:].rearrange("p (m c) -> p m c", c=ST)[:, :, HD:ST]
                nc.vector.memset(v1g, 1.0)

            # startup loads: interleave wt/x piecewise so the first
            # projection k-steps can begin as soon as their slices land
            # (the DMA pipe serializes transfers; mega-DMAs are all-or-nothing)
            wt_r = wt_d.rearrange("(k p) c -> p k c", p=KT)
            wts_r = wts_t[:].rearrange("p (k c) -> p k c", c=384)
            xt0_t = px.tile([KT, NKT * SQC], BF16, tag="xt", name="xtc_0")
            xt0_r = xt0_t[:].rearrange("p (k c) -> p k c", c=SQC)
            xt_r = xt_d.rearrange("(k p) c -> p k c", p=KT)
            nc.scalar.dma_start(out=wts_r[:, 0:2], in_=wt_r[:, 0:2])
            nc.sync.dma_start(out=xt0_r[:, 0:2], in_=xt_r[:, 0:2, 0:SQC])
            nc.scalar.dma_start(out=wts_r[:, 2:5], in_=wt_r[:, 2:5])
            nc.sync.dma_start(out=xt0_r[:, 2:5], in_=xt_r[:, 2:5, 0:SQC])
            nc.scalar.dma_start(out=wts_r[:, 5:8], in_=wt_r[:, 5:8])
            nc.sync.dma_start(out=xt0_r[:, 5:8], in_=xt_r[:, 5:8, 0:SQC])
            xtc_map[0] = [xt0_t[:, k * SQC:(k + 1) * SQC] for k in range(NKT)]
            v1g0 = v1c[0][:].rearrange("p (m c) -> p m c", c=ST)[:, :, HD:ST]
            nc.vector.memset(v1g0, 1.0)
            # initialize the score ring: merged diag exps read whole halves,
            # including columns no scores matmul has written yet
            nc.vector.memset(pscrh[0][:], 0.0)
            nc.vector.memset(pscrh[1][:], 0.0)
            nc.scalar.dma_start(out=epst[:], in_=eps_d[:])
            nc.scalar.dma_start(out=gains[:], in_=gains_d[:])
            nc.scalar.dma_start(out=cosd[:], in_=cosd_d[:])
            nc.scalar.dma_start(out=sind[:], in_=sind_d[:])
            nc.scalar.dma_start(out=maskt[:], in_=mask_d[:])
            nc.scalar.dma_start(
                out=wotp_t[:].rearrange("p (t c) -> p t c", c=D),
                in_=wot_d.rearrange("(t p) c -> p t c", p=ST),
            )

            # ---------------- emission helpers ----------------
            # fq1: phase-1 ops (must drain before the next chunk's heads)
            # fq3: wo/output ops (can spill across chunks)
            fq1 = deque()
            fq3 = deque()
            flip = [0]

            def fill(n=1):
                for _ in range(n):
                    flip[0] += 1
                    qa, qb = (fq3, fq1) if flip[0] % 2 == 0 else (fq1, fq3)
                    if qa:
                        qa.popleft()()
                    elif qb:
                        qb.popleft()()
                    else:
                        return

            def drain_p1():
                while fq1:
                    fq1.popleft()()

            def drain_all():
                drain_p1()
                while fq3:
                    fq3.popleft()()

            # ---- phase 1: projection + stats + rope + transposes for a chunk
            def p1_ops(qc):
                """Return list of emission closures for phase-1 of chunk qc."""
                ops = []
                xtc = xtc_map  # read lazily; xtc[qc] created by emit_xtc op
                ss = pw_.tile([ST, 32], F32, tag="ss", name=f"ss{qc}")
                rr = pw_.tile([ST, 32], F32, tag="rr", name=f"rr{qc}")
                rg = pw_.tile([ST, 32], F32, tag="rg", name=f"rg{qc}")
                qkes = []

                def proj_mm(s, k0):
                    def op():
                        _lbl(f"proj{qc}.{s}.{k0}")
                        if k0 == 0:
                            if qc == 0 and s % 2 == 1:
                                proj = ps_po.tile([ST, 384], F32, tag="po",
                                                  name=f"pj{qc}_{s}",
                                                  padded_shape=[ST, SQC])
                            else:
                                proj = ps_b0.tile([ST, 384], F32, tag="b0",
                                                  name=f"pj{qc}_{s}",
                                                  padded_shape=[ST, SQC])
                            qkes.append(proj)
                        proj = qkes[s]
                        for k in range(k0, k0 + 2):
                            nc.tensor.matmul(
                                proj[:],
                                xtc[qc][k][:, s * ST:(s + 1) * ST],
                                wts[k][:],
                                start=(k == 0),
                                stop=(k == NKT - 1),
                            )
                    op.is_pe = True
                    return op

                qke_t = []

                def evac(s):
                    def op():
                        _lbl(f"evac{qc}.{s}")
                        proj = qkes[s]
                        qke = pw_.tile([ST, 384], BF16, tag="qke", name=f"qke{qc}_{s}",
                                       bufs=8)
                        qke_t.append(qke)
                        if qc <= 1:
                            # prologue: ACT is idle until the first exp
                            nc.scalar.activation(qke[:], proj[:], AF.Copy)
                        else:
                            nc.vector.tensor_copy(qke[:], proj[:])
                        nc.gpsimd.tensor_copy(
                            v1c[qc][:, s * ST:s * ST + HD],
                            qke[:, 320:384],
                        )
                    return op

                def stats(s):
                    def op():
                        _lbl(f"stats{qc}.{s}")
                        qke = qke_t[s]
                        sq = pw_.tile([ST, 320], BF16, tag="sq", name=f"sq{qc}_{s}")
                        nc.gpsimd.tensor_tensor(sq[:], qke[:, 0:320], qke[:, 0:320],
                                                AL.mult)
                        nc.vector.tensor_reduce(
                            ss[:, 8 * s:8 * s + NH5],
                            sq[:].rearrange("p (h d) -> p h d", d=HD),
                            axis=mybir.AxisListType.X,
                            op=AL.add,
                        )
                    return op

                def chunk_stats():
                    _lbl(f"cstats{qc}")
                    # rstd*gain = exp(-0.5*ln(ms+eps)) * gain, batched over 4
                    # stiles. High priority: these two tiny ACT ops gate the
                    # whole rope -> transpose chain and must not queue behind
                    # attention exps.
                    lg = pw_.tile([ST, 32], F32, tag="lg", name=f"lg{qc}")
                    sv = ss[:].rearrange("p (s c) -> p s c", c=8)[:, :, 0:NH5]
                    lv = lg[:].rearrange("p (s c) -> p s c", c=8)[:, :, 0:NH5]
                    rv = rr[:].rearrange("p (s c) -> p s c", c=8)[:, :, 0:NH5]
                    gv = rg[:].rearrange("p (s c) -> p s c", c=8)[:, :, 0:NH5]
                    with tc.high_priority(offset=400):
                        nc.scalar.activation(lv, sv, AF.Ln, bias=epst[:, 0:1],
                                             scale=1.0 / HD)
                        nc.scalar.activation(rv, lv, AF.Exp, scale=-0.5)
                        nc.vector.tensor_tensor(
                            gv, rv,
                            gains[:, 0:NH5].unsqueeze(1).broadcast_to([ST, 4, NH5]),
                            AL.mult,
                        )

                def rope_a(s):
                    def op():
                        _lbl(f"rope{qc}.{s}")
                        m = qc * CT + s
                        qke = qke_t[s]
                        qke3 = qke[:, 0:320].rearrange("p (h d) -> p h d", d=HD)
                        cosm = cosd[:, m * HD:(m + 1) * HD]
                        sinm = sind[:, m * HD:(m + 1) * HD]
                        tcc = pw_.tile([ST, 320], BF16, tag="tcc", name=f"tc{qc}_{s}")
                        nc.vector.tensor_tensor(
                            tcc[:].rearrange("p (h d) -> p h d", d=HD),
                            qke3,
                            cosm.unsqueeze(1).broadcast_to([ST, NH5, HD]),
                            AL.mult,
                        )
                        tss = pw_.tile([ST, 320], BF16, tag="tss", name=f"ts{qc}_{s}")
                        tss3 = tss[:].rearrange("p (h d) -> p h d", d=HD)
                        HH = HD // 2
                        nc.vector.tensor_tensor(
                            tss3[:, :, 0:HH],
                            qke3[:, :, HH:HD],
                            sinm[:, 0:HH].unsqueeze(1).broadcast_to([ST, NH5, HH]),
                            AL.mult,
                        )
                        nc.vector.tensor_tensor(
                            tss3[:, :, HH:HD],
                            qke3[:, :, 0:HH],
                            sinm[:, HH:HD].unsqueeze(1).broadcast_to([ST, NH5, HH]),
                            AL.mult,
                        )
                        qkrr = pw_.tile([ST, 320], BF16, tag="qkrr", name=f"qr{qc}_{s}")
                        nc.vector.tensor_tensor(qkrr[:], tcc[:], tss[:], AL.add)
                        qkr = pw_.tile([ST, 320], BF16, tag="qkr", name=f"qk{qc}_{s}")
                        nc.vector.tensor_tensor(
                            qkr[:, 0:320].rearrange("p (h d) -> p h d", d=HD),
                            qkrr[:].rearrange("p (h d) -> p h d", d=HD),
                            rg[:, 8 * s:8 * s + NH5].unsqueeze(2)
                              .broadcast_to([ST, NH5, HD]),
                            AL.mult,
                        )
                        qkr_t.append(qkr)
                    return op

                qkr_t = []

                def tr_dma(s, i):
                    def op():
                        _lbl(f"trq{qc}.{s}.{i}")
                        qkr = qkr_t[s]
                        c0 = (0, 128, 192)[i]
                        nc.sync.dma_start(
                            out=qtT[qc][i][:, s * ST:(s + 1) * ST],
                            in_=qkr[:, c0:c0 + ST],
                            transpose=True,
                        )
                    return op

                for s in range(CT):
                    for k0 in range(0, NKT, 2):
                        ops.append(proj_mm(s, k0))
                    ops.append(evac(s))
                    ops.append(stats(s))
                ops.append(chunk_stats)
                # per-stile A/D transposes fire immediately after their own
                # rope (head0 needs A+D); C then B follow for heads 1-3.
                # HWDGE generation is serialized, so issue order = first use.
                for s in range(CT):
                    ops.append(rope_a(s))
                    ops.append(tr_dma(s, 0))
                    ops.append(tr_dma(s, 2))
                for s in range(CT):
                    ops.append(tr_dma(s, 1))
                return ops

            # ---- phase 3: wo matmuls + evac + output DMA ----
            # chunks 0..2: both head-pairs accumulated into one psum (part0).
            # final chunk: split by head-pair t so t=0 streams after head 1;
            # the t=1 tail rotates through the freed po banks (part1).
            wo_po_alt = [0]

            def p3_ops(qc, mode):
                if mode == "t1":
                    wo_po_alt[0] = 0
                ops = []

                obs = {}

                def half(s, nch, ts, part_t, use_po):
                    def op():
                        _lbl(f"wo{qc}.{s}.{nch}.{mode}")
                        m = qc * CT + s
                        if use_po and wo_po_alt[0] % 3 != 0:
                            pw = ps_po.tile([ST, SQC], F32, tag="po",
                                            name=f"pwp{qc}_{s}_{nch}")
                        else:
                            pw = ps_w.tile([ST, SQC], F32, tag="pw",
                                           name=f"pw{qc}_{s}_{nch}")
                        if use_po:
                            wo_po_alt[0] += 1
                        for i, t in enumerate(ts):
                            nc.tensor.matmul(
                                pw[:],
                                ypc[t][qc][:, s * ST:(s + 1) * ST],
                                wotp[t][:, nch * SQC:(nch + 1) * SQC],
                                start=(i == 0),
                                stop=(i == len(ts) - 1),
                            )
                        if nch == 0:
                            ob = pob.tile([ST, D], BF16, tag="ob",
                                          name=f"ob{qc}_{s}_{mode}", bufs=4)
                            obs[s] = ob
                        else:
                            ob = obs[s]
                        o0 = nch * SQC
                        if mode == "both":
                            nc.vector.tensor_copy(ob[:, o0:o0 + SQC], pw[:])
                        else:
                            nc.vector.tensor_copy(ob[:, o0:o0 + 256], pw[:, 0:256])
                            nc.scalar.activation(ob[:, o0 + 256:o0 + SQC],
                                                 pw[:, 256:SQC], AF.Copy)
                        if nch == 1:
                            nc.sync.dma_start(
                                out=part_t[m * ST:(m + 1) * ST, :],
                                in_=ob[:],
                            )
                    op.is_pe = True
                    return op

                if mode == "both":
                    ts, part_t, use_po = (0, 1), part0_d, False
                elif mode == "t0":
                    ts, part_t, use_po = (0,), part0_d, False
                else:
                    ts, part_t, use_po = (1,), part1_d, True
                for s in range(CT):
                    ops.append(half(s, 0, ts, part_t, use_po))
                    ops.append(half(s, 1, ts, part_t, use_po))
                return ops

            # ---- attention head ----
            ring = [0]   # score-ring cursor (4 psum slots; pairs aligned)
            aring = [0]  # atr-half cursor (3 sbuf halves)

            def emit_attn_head(qc, h):
                nsk = (qc + 1) * CT
                if qc == 0 and h == 2:
                    # pw bank is idle before any wo work: extra po slot
                    po = ps_w.tile([ST, SQC], F32, tag="pw", name=f"po{qc}_{h}")
                else:
                    po = ps_po.tile([ST, SQC], F32, tag="po", name=f"po{qc}_{h}")
                qT = qTc[h][qc]
                info = []
                for sk in range(nsk):
                    slot = ring[0]
                    ring[0] = (ring[0] + 1) % 4
                    dj = sk - qc * CT
                    cb = dj * ST if dj >= 0 else 0
                    info.append((slot, cb))
                pair_af = []
                for p in range(nsk // 2):
                    pair_af.append(aring[0])
                    aring[0] = (aring[0] + 1) % 3

                def scores(sk):
                    _lbl(f"sc{qc}.{h}.{sk}")
                    slot, cb = info[sk]
                    hf, sub = slot // 2, slot % 2
                    skc, skm = sk // CT, sk % CT
                    # kT view whose base partition matches qT's (h parity)
                    kT = kTc[skc][h % 2]
                    nc.tensor.matmul(
                        pscrh[hf][:, sub * SQC + cb:(sub + 1) * SQC],
                        kT[:, skm * ST:(skm + 1) * ST],
                        qT[:, cb:SQC],
                        start=True,
                        stop=True,
                    )

                def exp_pair(p):
                    _lbl(f"exp{qc}.{h}.{p}")
                    sk0 = 2 * p
                    hf = info[sk0][0] // 2
                    af = pair_af[p]
                    diag = sk0 >= qc * CT
                    if not diag or qc <= 3:
                        # one exp over the whole ring half; for diag pairs the
                        # left-of-cb columns hold stale-but-bounded scores and
                        # are never read by the trimmed AV (chunk 3 keeps the
                        # trimmed singles: ACT is the pacer there)
                        nc.scalar.activation(
                            atrh[af][:], pscrh[hf][:], AF.Exp,
                        )
                        if diag:
                            for sk in (sk0, sk0 + 1):
                                slot, cb = info[sk]
                                sub = slot % 2
                                with tc.high_priority(offset=300):
                                    nc.vector.tensor_tensor(
                                        atrh[af][:, sub * SQC + cb:sub * SQC + cb + ST],
                                        atrh[af][:, sub * SQC + cb:sub * SQC + cb + ST],
                                        maskt[:],
                                        AL.mult,
                                    )
                    else:
                        for sk in (sk0, sk0 + 1):
                            slot, cb = info[sk]
                            sub = slot % 2
                            nc.scalar.activation(
                                atrh[af][:, sub * SQC + cb:(sub + 1) * SQC],
                                pscrh[hf][:, sub * SQC + cb:(sub + 1) * SQC],
                                AF.Exp,
                            )
                            with tc.high_priority(offset=300):
                                nc.vector.tensor_tensor(
                                    atrh[af][:, sub * SQC + cb:sub * SQC + cb + ST],
                                    atrh[af][:, sub * SQC + cb:sub * SQC + cb + ST],
                                    maskt[:],
                                    AL.mult,
                                )

                def av(sk):
                    _lbl(f"av{qc}.{h}.{sk}")
                    slot, cb = info[sk]
                    sub = slot % 2
                    af = pair_af[sk // 2]
                    skc, skm = sk // CT, sk % CT
                    nc.tensor.matmul(
                        po[:, cb:SQC],
                        v1c[skc][:, skm * ST:(skm + 1) * ST],
                        atrh[af][:, sub * SQC + cb:(sub + 1) * SQC],
                        start=(sk == 0),
                        stop=(sk == nsk - 1),
                    )

                # software-pipelined: scores run two pairs ahead so exp(p+1)'s
                # inputs complete during exp(p) and ACT goes back-to-back.
                # s(2p+4) reuses exp(p)'s input slot, so it must be emitted
                # after exp_pair(p) (program order creates the WAR dep).
                for sk in range(min(4, nsk)):
                    scores(sk)
                    fill(1)
                for p in range(nsk // 2):
                    exp_pair(p)
                    if 2 * p + 4 < nsk:
                        scores(2 * p + 4)
                        fill(1)
                        scores(2 * p + 5)
                        fill(1)
                    av(2 * p)
                    fill(1)
                    av(2 * p + 1)
                    fill(1)
                # tail: po rows 64:128 all hold the denominator (v1 ones
                # half); reciprocal over the replica = broadcast 1/den.
                # high priority: frees the po slot for head h+2 sooner.
                _lbl(f"tail{qc}.{h}")
                rb = pw_.tile([HD, SQC], F32, tag="rb", name=f"rb{qc}_{h}")
                with tc.high_priority(offset=200):
                    nc.vector.reciprocal(rb[:], po[HD:ST, :])
                    t, r = h // 2, (h % 2) * HD
                    nc.vector.tensor_tensor(
                        ypc[t][qc][r:r + HD, :], po[0:HD, :], rb[:], AL.mult
                    )

            # ---------------- main emission ----------------
            for rep in range(reps):
                # chunk 0 phase 1, emitted directly (nothing to interleave
                # with); chunk 1's projection/stats stages follow eagerly so
                # the b0-slot chain overlaps chunk-0's rope instead of its
                # attention, leaving rope/transposes as filler
                for op in p1_ops(0):
                    op()
                emit_xtc(1)
                o1 = p1_ops(1)
                for op in o1[:19]:
                    op()
                for op in o1[19:]:
                    fq1.append(op)
                for qc in range(NSQC):
                    if 2 <= qc + 1 < NSQC:
                        emit_xtc(qc + 1)
                        for op in p1_ops(qc + 1):
                            fq1.append(op)
                    if qc >= 1:
                        for op in p3_ops(qc - 1, "both"):
                            fq3.append(op)
                    last = qc == NSQC - 1
                    for h in range(G):
                        emit_attn_head(qc, h)
                        if last and h == 1:
                            for op in p3_ops(qc, "t0"):
                                fq3.append(op)
                        elif last and h == 3:
                            for op in p3_ops(qc, "t1"):
                                fq3.append(op)
                        # keep phase-1 filler on pace across the chunk's heads
                        heads_left = G - h - 1
                        if heads_left:
                            target = (len(fq1) + heads_left - 1) // heads_left
                            extra = len(fq1) - target * heads_left
                            while extra > 0 and fq1:
                                fq1.popleft()()
                                extra -= 1
                        else:
                            drain_p1()
                drain_all()
    return nc


def _host_tables():
    inv_freq = 1.0 / (ROPE_BASE ** (np.arange(0, HD, 2, dtype=np.float32) / HD))
    pos = np.arange(S, dtype=np.float32)
    freqs = np.outer(pos, inv_freq)  # [S, 32]
    cos = np.cos(freqs).astype(np.float32)
    sin = np.sin(freqs).astype(np.float32)
    cosdup = np.concatenate([cos, cos], axis=1)        # [S, 64]
    sindup = np.concatenate([sin, -sin], axis=1)       # [S, 64]
    # rearrange [S, 64] -> [128, 16*64] with [p, m*64+j] = v[m*128+p, j]
    def arr(v):
        return np.ascontiguousarray(
            v.reshape(NST, ST, HD).transpose(1, 0, 2).reshape(ST, NST * HD)
        )
    mask = np.tril(np.ones((ST, ST), np.float32)).T  # mask[i,j] = 1 iff i<=j
    return arr(cosdup), arr(sindup), mask


_NC_CACHE = None


def _get_nc():
    global _NC_CACHE
    if _NC_CACHE is None:
        nc = build_nc()
        _split_waits(nc, cap=1)
        _NC_CACHE = nc
    return _NC_CACHE


BF = ml_dtypes.bfloat16


def make_in_maps(x, wq, wk, wv, wo, q_gain):
    x = np.asarray(x, np.float32)
    wq, wk, wv, wo = (np.asarray(a, np.float32) for a in (wq, wk, wv, wo))
    q_gain = np.asarray(q_gain, np.float32)
    cosd, sind, mask = _host_tables()
    epsc = np.full((ST, 1), EPS, np.float32)
    in_maps = []
    for c in range(NC):
        b, g = c // KVH, c % KVH
        xT = np.ascontiguousarray(x[b].T)  # [D, S]
        wq_c = wq[256 * g:256 * (g + 1), :]
        wk_c = wk[HD * g:HD * (g + 1), :]
        wv_c = wv[HD * g:HD * (g + 1), :]
        # column layout [q0 q1 q2 | k | q3 | v]: k sits in the overlap of the
        # two upper transposes so it lands at both base partitions
        wt = np.ascontiguousarray(
            np.concatenate([wq_c[0:192].T, wk_c.T, wq_c[192:256].T, wv_c.T],
                           axis=1)
        )  # [D, 384]
        wot = np.ascontiguousarray(wo[:, 256 * g:256 * (g + 1)].T)  # [256, D]
        gains = np.zeros((ST, 8), np.float32)
        qg = q_gain[G * g:G * (g + 1)] / np.sqrt(HD)
        gains[:, 0:3] = qg[None, 0:3]
        gains[:, 3] = 1.0       # k segment
        gains[:, 4] = qg[3]
        in_maps.append(
            dict(
                xt=xT.astype(BF), wt=wt.astype(BF), wot=wot.astype(BF),
                cosd=cosd.astype(BF), sind=sind.astype(BF), gains=gains,
                mask=mask.astype(BF), epsc=epsc,
            )
        )
    return in_maps


def kernel(x, wq, wk, wv, wo, q_gain):
    global LAST_EXEC_NS
    nc = _get_nc()
    in_maps = make_in_maps(x, wq, wk, wv, wo, q_gain)
    trace = os.environ.get("BASS_KERNEL_TRACE", "") == "1"
    r = run_bass_kernel_spmd(nc, in_maps, list(range(NC)), trace=trace)
    LAST_EXEC_NS = r.exec_time_ns
    parts = []
    for c in range(NC):
        p = np.asarray(r.results[c]["part0"]).astype(np.float32)
        p[-SQC:, :] += np.asarray(
            r.results[c]["part1"][-SQC:, :]).astype(np.float32)
        parts.append(p)
    out = np.stack(
        [sum(parts[0:KVH]), sum(parts[KVH:2 * KVH])], axis=0
    ).astype(np.float32)
    return out
